# revision 20
# baseline (speedup 1.0000x reference)
"""Trainium2 Bass kernel: single-head attention (B=4, S=2048, D=1024) on 8 NeuronCores.

Sharding: data-parallel over (batch, query-half): core c handles batch c//2,
query rows [c%2*1024, (c%2+1)*1024), and (dist mode) computes the K-projection
only for its own kv half; the pair {2b, 2b+1} exchanges K^T via a 2-rank
AllGather overlapped with the Q-projection and the local-half attention.

Math per core (all matmuls bf16, fp32 PSUM accumulation):
  QT[dk,q]  = Wq(lhsT) . XqT(rhs)                 (+bq)
  KT[dk,s]  = Wk(lhsT) . XkvT(rhs)                (+bk)   [local half, AG for rest]
  sT[s,q]   = KT-tiles(lhsT) . QT(rhs)            scores^T
  eT[s,q]   = exp(sT / sqrt(dk) [+ maskT])        ScalarE, PSUM->SBUF bf16
  sums[1,q] = ones(lhsT) . eT(rhs)                softmax denominators
  HT[dm,q]  = Xkv-tiles(lhsT) . eT(rhs)           H = probs_unnorm @ Xkv
  out[q,dv] = HT-tiles(lhsT) . Wv(rhs)  (+bv)     (probs@Xkv)@Wv == probs@(Xkv@Wv)
  out      *= 1/sums  (per-partition scale on ScalarE, fused with PSUM->SBUF copy)

DMA design (from trace analysis):
- Each hw queue serves ~one dma_start at a time with multi-us turnaround, so
  aggregate startup bandwidth scales with in-flight dma_starts: keep payloads
  ~128-256KB and fan out across all ring slots of all 3 DMA-capable engines.
- A dma_start's wait condition blocks the ISSUING ENGINE's stream, so: the
  scalar engine (which runs every PSUM->SBUF activation) gets only a handful
  of pre-activation issues; collective readbacks (which wait on the AllGather)
  go on sync+gpsimd after everything else those engines must issue first.
- wq/wk are host-packed d_key-block-major, xqt/xkvt chunk-tile-major, and xkv
  (j, s)-tile-major, so every DMA has >=2KB contiguous segments and each PE
  accumulation group depends on the minimum payload (first matmul needs only
  ~0.5MB: wk block 0 + the leading 128-col xkvt chunk).

dist mode details: the s axis lives in LOCAL-relative order on each core
(m-tiles 0..7 = own kv half, 8..15 = partner's). The host feeds xkv/maskt
pre-permuted to match. The AllGather output is rank-ordered (identical layout
on both cores), so the partner block is recovered SPMD-uniformly via the exact
bf16-bit identity  remote = g0 XOR g1 XOR local  on uint32 views (own block
round-trips bit-identically through the collective).
"""

import os
import numpy as np
import ml_dtypes

B, S, D = 4, 2048, 1024
N_CORES = 8
QL = S // 2  # query rows per core (1024)
BF16 = ml_dtypes.bfloat16

_cache: dict = {}


def _kch(kvl):
    # K-projection chunk widths: narrow leading chunks shrink the
    # startup-critical DMA payload
    lead = int(os.environ.get("KERNEL_KCH", "256"))
    ch = [512] if lead == 512 else [lead, lead]
    while sum(ch) < kvl:
        ch.append(512)
    return ch


def _build(dist: bool, with_mask: bool, with_bq: bool, with_bk: bool,
           with_bv: bool, ps_bufs: int = 4):
    import concourse.bass as bass
    import concourse.mybir as mybir
    import concourse.tile as tile
    from concourse import bacc

    fp32 = mybir.dt.float32
    bf16 = mybir.dt.bfloat16
    uint32 = mybir.dt.uint32

    nc = bacc.Bacc("TRN2", target_bir_lowering=False, debug=False,
                   num_devices=N_CORES)

    KVL = QL if dist else S  # kv rows projected locally
    NT_D = D // 128    # 8 tiles along d_model / d_key
    NT_S = S // 128    # 16 tiles along s
    NT_L = NT_S // 2   # 8 (local-half s tiles in dist mode)
    NQ = QL // 512     # 2 query chunks of 512
    NV = D // 512      # 2 dv chunks of 512
    SCALE = 1.0 / float(np.sqrt(D))

    KCH = _kch(KVL)
    KOFF = [sum(KCH[:c]) for c in range(len(KCH))]
    LAST_C = len(KCH) - 1

    # all activations host-packed into SBUF tile layouts (fully-contiguous
    # DMA segments); wq/wk packed d_key-block-major
    xqt_d = nc.dram_tensor("xqt", (128, NT_D * QL), bf16, kind="ExternalInput")
    xkvt_d = nc.dram_tensor("xkvt", (128, NT_D * KVL), bf16,
                            kind="ExternalInput")
    xkv_d = nc.dram_tensor("xkv", (128, NT_D * S), bf16, kind="ExternalInput")
    wq_d = nc.dram_tensor("wq", (128, NT_D * D), bf16, kind="ExternalInput")
    wk_d = nc.dram_tensor("wk", (128, NT_D * D), bf16, kind="ExternalInput")
    wv_d = nc.dram_tensor("wv", (D, D), bf16, kind="ExternalInput")
    if with_bq:
        bq_d = nc.dram_tensor("bq", (128, 8), fp32, kind="ExternalInput")
    if with_bk:
        bk_d = nc.dram_tensor("bk", (128, 8), fp32, kind="ExternalInput")
    if with_bv:
        bv_d = nc.dram_tensor("bv", (1, D), bf16, kind="ExternalInput")
    if with_mask:
        maskt_d = nc.dram_tensor("maskt", (S, QL), bf16, kind="ExternalInput")
    out_d = nc.dram_tensor("out", (QL, D), bf16, kind="ExternalOutput")

    if dist:
        # the K^T exchange is split into two AllGathers (d_key rows 0:512 and
        # 512:1024) so the first one triggers as soon as the first half of the
        # K-projection lands — the readback pipeline starts ~20us earlier
        cc_ins = [nc.dram_tensor(f"cc_in{h}", (D // 2, QL), bf16)
                  for h in range(2)]
        cc_outs = [nc.dram_tensor(f"cc_out{h}", (D, QL), bf16)
                   for h in range(2)]
        groups = [[2 * g, 2 * g + 1] for g in range(4)]

    with tile.TileContext(nc) as tc:
        with (
            tc.tile_pool(name="cons", bufs=1) as cons,
            tc.tile_pool(name="wkb", bufs=1) as wkbp,
            tc.tile_pool(name="wqb", bufs=1) as wqbp,
            tc.tile_pool(name="big", bufs=3) as bigp,
            tc.tile_pool(name="kt", bufs=1) as ktp,
            tc.tile_pool(name="et", bufs=1) as etp,
            tc.tile_pool(name="xk", bufs=1) as xkp,
            tc.tile_pool(name="xq", bufs=2) as xqp,
            tc.tile_pool(name="xkv", bufs=4) as xkvp,
            tc.tile_pool(name="gch", bufs=2) as gchp,
            tc.tile_pool(name="outp", bufs=2) as outp,
            tc.tile_pool(name="mask", bufs=2) as maskp,
            tc.tile_pool(name="ps", bufs=ps_bufs,
                         space=bass.MemorySpace.PSUM) as psp,
            tc.tile_pool(name="pss", bufs=2, space=bass.MemorySpace.PSUM) as pssp,
            tc.tile_pool(name="pst", bufs=1, space=bass.MemorySpace.PSUM) as pstp,
        ):
            # ---- startup-critical DMA wave ----
            wk_blks = [wkbp.tile([128, D], bf16, tag=f"wkb{i}",
                                 name=f"wkb{i}")
                       for i in range(NT_D)]
            xk_chs = [xkp.tile([128, NT_D, KCH[c]], bf16,
                               tag=f"xk{c}", name=f"xk{c}")
                      for c in range(len(KCH))]

            def ld_wk(eng, i):   # one i-block: 256KB
                eng.dma_start(wk_blks[i][:],
                              wk_d.ap()[:, i * D:(i + 1) * D])

            def ld_xk(eng, c, part, nparts):  # 1/nparts of chunk c
                w = KCH[c]
                jn = NT_D // nparts
                base = NT_D * KOFF[c]
                eng.dma_start(
                    xk_chs[c][:, part * jn:(part + 1) * jn, :],
                    xkvt_d.ap()[:, base + part * jn * w:
                                base + (part + 1) * jn * w])

            sy, sc, gp = nc.sync, nc.scalar, nc.gpsimd
            # (engine, payload) in consumer-priority order. ~256-512KB per
            # dma_start: few enough issue ops (~0.65us each) that the whole
            # 4MB critical set is in flight by ~12us, big enough to keep the
            # queue ring slots saturated. scalar gets only 2 issues so its
            # first K activation isn't delayed.
            ld_wk(sy, 0); ld_xk(sc, 0, 0, 2); ld_xk(gp, 0, 1, 2)
            ld_wk(sy, 1); ld_wk(sc, 2); ld_wk(gp, 3)
            ld_xk(sy, 1, 0, 2); ld_xk(gp, 1, 1, 2)
            ld_wk(sy, 4); ld_wk(gp, 5)
            ld_xk(sy, 2, 0, 2); ld_xk(gp, 2, 1, 2)
            ld_wk(sy, 6); ld_wk(gp, 7)
            for c in range(3, len(KCH)):
                ld_xk(sy, c, 0, 2); ld_xk(gp, c, 1, 2)

            # ---- Q-stage loads (needed ~28us later): sync + gpsimd ----
            wq_blks = [wqbp.tile([128, D], bf16, tag=f"wqb{i}",
                                 name=f"wqb{i}")
                       for i in range(NT_D)]
            xq_chs = [xqp.tile([128, NT_D, 512], bf16, tag="xq",
                               name=f"xq{n}")
                      for n in range(NQ)]

            def ld_xq(eng, n, part):
                eng.dma_start(
                    xq_chs[n][:, part * 4:(part + 1) * 4, :],
                    xqt_d.ap()[:, n * NT_D * 512 + part * 2048:
                               n * NT_D * 512 + (part + 1) * 2048])

            ld_xq(sy, 0, 0); ld_xq(gp, 0, 1)
            for i in range(NT_D):
                (sy if i % 2 == 0 else gp).dma_start(
                    wq_blks[i][:], wq_d.ap()[:, i * D:(i + 1) * D])
            ld_xq(sy, 1, 0); ld_xq(gp, 1, 1)

            # wv on gpsimd BEFORE the collective trigger (which blocks the
            # gpsimd stream until the cc_in stores land)
            wv_sb = bigp.tile([128, NT_D, D], bf16, tag="big", name="wv_sb")
            for h in range(4):
                nc.gpsimd.dma_start(
                    wv_sb[:, 2 * h:2 * h + 2, :],
                    wv_d.ap()[2 * h * 128:(2 * h + 2) * 128, :]
                    .rearrange("(j p) d -> p j d", p=128))

            # ---- constants / biases (tiny, off the critical path) ----
            ones_col = cons.tile([128, 1], fp32, tag="ones_col")
            nc.gpsimd.memset(ones_col[:], 1.0)
            ident1 = cons.tile([1, 1], fp32, tag="ident1")
            nc.gpsimd.memset(ident1[:], 1.0)
            if with_bv:
                bv_sb = cons.tile([1, D], bf16, tag="bv")
                nc.sync.dma_start(bv_sb[:], bv_d.ap()[:])
            if with_bq:
                bq_sb = cons.tile([128, 8], fp32, tag="bq")
                nc.sync.dma_start(bq_sb[:], bq_d.ap()[:])
            if with_bk:
                bk_sb = cons.tile([128, 8], fp32, tag="bk")
                nc.sync.dma_start(bk_sb[:], bk_d.ap()[:])

            kt_sb = ktp.tile([128, NT_D, KVL], bf16, tag="kt")
            if dist:
                ktr_sb = ktp.tile([128, NT_D, QL], bf16, tag="ktr")
            et_sb = etp.tile([128, NT_S, QL], bf16, tag="et")

            # xkv prefetch helper (gpsimd): packed (j, s)-tile-major layout,
            # one contiguous 256KB dma_start per (pass, j)
            xkv_chunks = {}

            def prefetch_xkv(ms, js):
                for j in js:
                    xkv_ch = xkvp.tile([128, len(ms), 128], bf16, tag="xkv",
                                       name=f"xkv{ms[0]}_{j}")
                    nc.gpsimd.dma_start(
                        xkv_ch[:],
                        xkv_d.ap()[:, j * NT_S * 128 + ms[0] * 128:
                                   j * NT_S * 128 + (ms[-1] + 1) * 128])
                    xkv_chunks[(j, ms[0])] = xkv_ch

            # first-half xkv for stage 3a: issue the first 4 (= pool depth)
            # before the collective occupies the gpsimd stream
            local_ms = list(range(NT_L)) if dist else list(range(NT_S))
            prefetch_xkv(local_ms, range(4))

            # ---- stage 1b: KT (local half in dist mode) -> kt_sb ----
            # half-outer/chunk-outer: d_key rows 0:512 finish first (feeding
            # AllGather #1 early) and the first groups need only chunk 0;
            # cc_in row-stores go out inline as each row block completes.
            for half in range(2):
                ilist = range(half * NT_D // 2, (half + 1) * NT_D // 2)
                for c in range(len(KCH)):
                    w, off = KCH[c], KOFF[c]
                    for i in ilist:
                        ps = psp.tile([128, 512], fp32, tag="ps")
                        for j in range(NT_D):
                            nc.tensor.matmul(
                                ps[:, :w],
                                wk_blks[i][:, j * 128:(j + 1) * 128],
                                xk_chs[c][:, j, :],
                                start=(j == 0), stop=(j == NT_D - 1))
                        if with_bk:
                            nc.scalar.activation(
                                kt_sb[:, i, off:off + w], ps[:, :w],
                                mybir.ActivationFunctionType.Identity,
                                bias=bk_sb[:, i:i + 1])
                        else:
                            nc.scalar.activation(
                                kt_sb[:, i, off:off + w], ps[:, :w],
                                mybir.ActivationFunctionType.Copy)
                        if dist and c == LAST_C:
                            (nc.sync if i % 2 == 0 else nc.scalar).dma_start(
                                cc_ins[half].ap()[(i % 4) * 128:
                                                  (i % 4 + 1) * 128, :],
                                kt_sb[:, i, 0:QL])
                if dist:
                    nc.gpsimd.collective_compute(
                        "AllGather", mybir.AluOpType.bypass,
                        replica_groups=groups,
                        ins=[cc_ins[half].ap()[:].opt()],
                        outs=[cc_outs[half].ap()[:].opt()],
                    )

            # ---- stage 1a: QT -> qt_sb ----
            qt_sb = bigp.tile([128, NT_D, D], bf16, tag="big", name="qt_sb")
            for n in range(NQ):
                for i in range(NT_D):
                    ps = psp.tile([128, 512], fp32, tag="ps")
                    for j in range(NT_D):
                        nc.tensor.matmul(
                            ps[:], wq_blks[i][:, j * 128:(j + 1) * 128],
                            xq_chs[n][:, j, :],
                            start=(j == 0), stop=(j == NT_D - 1))
                    if with_bq:
                        nc.scalar.activation(
                            qt_sb[:, i, n * 512:(n + 1) * 512], ps[:],
                            mybir.ActivationFunctionType.Identity,
                            bias=bq_sb[:, i:i + 1])
                    else:
                        nc.scalar.activation(
                            qt_sb[:, i, n * 512:(n + 1) * 512], ps[:],
                            mybir.ActivationFunctionType.Copy)

            if dist:
                # read the gathered pair back, recover the partner's block via
                # remote = g0 ^ g1 ^ local (exact bf16 bit identity) -> ktr_sb.
                # sync + gpsimd (both are past their other issue duties; the
                # collective-wait embedded in these DMAs blocks the engine,
                # so they must never sit on the scalar stream)
                for i in range(NT_D):
                    cc_view = cc_outs[i // 4].ap().rearrange(
                        "(b r) f -> r b f", b=2)
                    r0 = (i % 4) * 128
                    g_ch = gchp.tile([128, 2, QL], bf16, tag="gch",
                                     name=f"gch{i}")
                    eng = nc.sync if i % 2 == 0 else nc.gpsimd
                    eng.dma_start(
                        g_ch[:, 0, :], cc_view[r0:r0 + 128, 0, :])
                    eng.dma_start(
                        g_ch[:, 1, :], cc_view[r0:r0 + 128, 1, :])
                    nc.vector.tensor_tensor(
                        g_ch[:, 0, :].bitcast(uint32),
                        g_ch[:, 0, :].bitcast(uint32),
                        g_ch[:, 1, :].bitcast(uint32),
                        mybir.AluOpType.bitwise_xor)
                    nc.vector.tensor_tensor(
                        ktr_sb[:, i, :].bitcast(uint32),
                        g_ch[:, 0, :].bitcast(uint32),
                        kt_sb[:, i, :].bitcast(uint32),
                        mybir.AluOpType.bitwise_xor)

            # ---- stage 2: scores^T + exp ----
            def score_group(m, n):
                kt, mm = (ktr_sb, m - NT_L) if (dist and m >= NT_L) else (kt_sb, m)
                ps = psp.tile([128, 512], fp32, tag="ps")
                for i in range(NT_D):
                    nc.tensor.matmul(
                        ps[:], kt[:, i, mm * 128:(mm + 1) * 128],
                        qt_sb[:, i, n * 512:(n + 1) * 512],
                        start=(i == 0), stop=(i == NT_D - 1))
                if with_mask:
                    mk = maskp.tile([128, 512], bf16, tag="mask")
                    nc.sync.dma_start(
                        mk[:], maskt_d.ap()[m * 128:(m + 1) * 128,
                                            n * 512:(n + 1) * 512])
                    nc.vector.tensor_tensor(
                        ps[:], ps[:], mk[:], mybir.AluOpType.add)
                nc.scalar.activation(
                    et_sb[:, m, n * 512:(n + 1) * 512], ps[:],
                    mybir.ActivationFunctionType.Exp, scale=SCALE)

            first_ms = range(NT_L) if dist else range(NT_S)
            for n in range(NQ):
                for m in first_ms:
                    score_group(m, n)

            # softmax denominators: accumulate expT tiles on the DVE (PE
            # has no slack; DVE has plenty). In-place fp32 chain.
            sacc = cons.tile([128, QL], fp32, tag="sacc")
            first_l = list(first_ms)
            nc.vector.tensor_tensor(
                sacc[:], et_sb[:, first_l[0], :], et_sb[:, first_l[1], :],
                mybir.AluOpType.add)
            for m in first_l[2:]:
                nc.vector.tensor_tensor(
                    sacc[:], sacc[:], et_sb[:, m, :], mybir.AluOpType.add)

            # ---- stage 3a: HT over available s-tiles ----
            ht_sb = bigp.tile([128, NT_D, D], bf16, tag="big", name="ht_sb")

            def ht_groups(ms, merge, skip_prefetch=0):
                prefetch_xkv(ms, range(skip_prefetch, NT_D))
                for j in range(NT_D):
                    xkv_ch = xkv_chunks[(j, ms[0])]
                    for n in range(NQ):
                        ps = psp.tile([128, 512], fp32, tag="ps")
                        for k2, m in enumerate(ms):
                            nc.tensor.matmul(
                                ps[:], xkv_ch[:, k2, :],
                                et_sb[:, m, n * 512:(n + 1) * 512],
                                start=(k2 == 0), stop=(k2 == len(ms) - 1))
                        dst = ht_sb[:, j, n * 512:(n + 1) * 512]
                        if merge:
                            nc.vector.tensor_tensor(
                                dst, ps[:], dst, mybir.AluOpType.add)
                        else:
                            nc.scalar.activation(
                                dst, ps[:],
                                mybir.ActivationFunctionType.Copy)

            if dist:
                ht_groups(local_ms, merge=False, skip_prefetch=4)
                # m-outer: exp(m) completes both q-chunks back-to-back so the
                # DVE sums chain below never lags the PE
                for m in range(NT_L, NT_S):
                    for n in range(NQ):
                        score_group(m, n)
                for m in range(NT_L, NT_S):
                    nc.vector.tensor_tensor(
                        sacc[:], sacc[:], et_sb[:, m, :], mybir.AluOpType.add)
                ht_groups(list(range(NT_L, NT_S)), merge=True)
            else:
                ht_groups(local_ms, merge=False, skip_prefetch=4)

            # sums[1, q]: single fp32 ones-matmul per q-chunk over sacc.
            # Allocated here; EMITTED inside stage 4 after the first output
            # group so the PE chews useful matmuls while the DVE chain ends.
            sums_sb = cons.tile([1, QL], fp32, tag="sums")
            pst = pstp.tile([128, 8], fp32, tag="pst")
            recip_sb = cons.tile([128, 8], fp32, tag="recip")
            if with_bv:
                sums_bf = cons.tile([1, QL], bf16, tag="sums_bf")

            def emit_sums():
                for n in range(NQ):
                    pss = pssp.tile([1, 512], fp32, tag="pss")
                    nc.tensor.matmul(
                        pss[:], ones_col[:], sacc[:, n * 512:(n + 1) * 512],
                        start=True, stop=True)
                    nc.scalar.activation(
                        sums_sb[:, n * 512:(n + 1) * 512], pss[:],
                        mybir.ActivationFunctionType.Copy)
                for p in range(8):
                    nc.tensor.transpose(
                        pst[:, p:p + 1], sums_sb[:, p * 128:(p + 1) * 128],
                        ident1[:])
                nc.vector.reciprocal(recip_sb[:], pst[:])
                if with_bv:
                    # out accumulates UNNORMALIZED; bias enters as sums[q]*bv
                    # so the final 1/sums scale leaves exactly +bv
                    nc.scalar.activation(sums_bf[:], sums_sb[:],
                                         mybir.ActivationFunctionType.Copy)

            # ---- stage 4: out = HT^T . Wv (+bv), normalized, bf16 out ----
            # p=0: matmuls first, then the sums block (PE stays busy while
            # the DVE chain finishes), then the p=0 normalization.
            for p in range(8):
                out_sb = outp.tile([128, D], bf16, tag="outsb")
                group_ps = []
                for n2 in range(NV):
                    ps = psp.tile([128, 512], fp32, tag="ps")
                    for j in range(NT_D):
                        nc.tensor.matmul(
                            ps[:], ht_sb[:, j, p * 128:(p + 1) * 128],
                            wv_sb[:, j, n2 * 512:(n2 + 1) * 512],
                            start=(j == 0),
                            stop=(j == NT_D - 1 and not with_bv))
                    group_ps.append(ps)
                if p == 0:
                    emit_sums()
                for n2, ps in enumerate(group_ps):
                    if with_bv:
                        nc.tensor.matmul(
                            ps[:], sums_bf[:, p * 128:(p + 1) * 128],
                            bv_sb[:, n2 * 512:(n2 + 1) * 512],
                            start=False, stop=True)
                    nc.scalar.activation(
                        out_sb[:, n2 * 512:(n2 + 1) * 512], ps[:],
                        mybir.ActivationFunctionType.Copy,
                        scale=recip_sb[:, p:p + 1])
                    if p == 7 and n2 == NV - 1:
                        # final chunk: split + both queues for a short drain
                        for hh in range(2):
                            lo = n2 * 512 + hh * 256
                            (nc.sync if hh == 0 else nc.scalar).dma_start(
                                out_d.ap()[p * 128:(p + 1) * 128, lo:lo + 256],
                                out_sb[:, lo:lo + 256])
                    else:
                        (nc.sync if (p + n2) % 2 == 0 else
                         nc.scalar).dma_start(
                            out_d.ap()[p * 128:(p + 1) * 128,
                                       n2 * 512:(n2 + 1) * 512],
                            out_sb[:, n2 * 512:(n2 + 1) * 512])

    nc.compile()
    return nc


def _get_nc(flags):
    if flags not in _cache:
        _cache[flags] = _build(*flags)
    return _cache[flags]


def _flags_of(inputs, dist=True):
    return _prep_in_maps(**inputs, dist=dist)[0]


def _pack_w(W):
    # [j*128+p, i*128+c] -> [p, i*1024 + j*128 + c]
    return np.ascontiguousarray(
        W.reshape(8, 128, 8, 128).transpose(1, 2, 0, 3).reshape(128, 8192))


def _pack_x(xt, widths):
    # xt [D, L] (row-major) -> [128, sum_c 8*w_c]: for each col-chunk c,
    # block[p, j*w + t] = xt[j*128+p, off+t]  (SBUF tile layout, so each
    # chunk is one fully-contiguous DMA)
    blocks = []
    off = 0
    for w in widths:
        blk = xt[:, off:off + w].reshape(8, 128, w).transpose(1, 0, 2)
        blocks.append(blk.reshape(128, 8 * w))
        off += w
    return np.ascontiguousarray(np.concatenate(blocks, axis=1))


def _pack_kv(xkv):
    # xkv [S, D] -> [128, j*S + m*128 + t] with xkv[m*128+p, j*128+t]:
    # per (s-pass, j) loads are fully contiguous
    nts = xkv.shape[0] // 128
    return np.ascontiguousarray(
        xkv.reshape(nts, 128, 8, 128).transpose(1, 2, 0, 3)
        .reshape(128, nts * 1024))


def _prep_in_maps(query_input, keyvalue_input, mask, Wq, bq, Wk, bk, Wv, bv,
                  dist=True):
    qi = np.asarray(query_input, np.float32)
    kv = np.asarray(keyvalue_input, np.float32)
    mask = np.asarray(mask, np.float32)
    Wqb = np.asarray(Wq, np.float32).astype(BF16)
    Wkb = np.asarray(Wk, np.float32).astype(BF16)
    Wvb = np.asarray(Wv, np.float32).astype(BF16)
    bq = np.asarray(bq, np.float32)
    bk = np.asarray(bk, np.float32)
    bv = np.asarray(bv, np.float32)

    with_mask = bool(np.any(mask != 0.0))
    with_bq = bool(np.any(bq != 0.0))
    with_bk = bool(np.any(bk != 0.0))
    with_bv = bool(np.any(bv != 0.0))
    flags = (dist, with_mask, with_bq, with_bk, with_bv)

    Wq_p = _pack_w(Wqb)
    Wk_p = _pack_w(Wkb)

    in_maps = []
    for c in range(N_CORES):
        b, h = c // 2, c % 2
        xq = qi[b, h * QL:(h + 1) * QL, :].astype(BF16)       # [QL, D]
        xkv = kv[b].astype(BF16)                               # [S, D]
        if dist:
            xkvt = np.ascontiguousarray(xkv[h * QL:(h + 1) * QL, :].T)
            perm_kv = np.concatenate(
                [xkv[h * QL:(h + 1) * QL], xkv[(1 - h) * QL:(2 - h) * QL]])
        else:
            xkvt = np.ascontiguousarray(xkv.T)
            perm_kv = xkv
        m = {
            "xqt": _pack_x(np.ascontiguousarray(xq.T), [512, 512]),
            "xkvt": _pack_x(xkvt, _kch(xkvt.shape[1])),
            "xkv": _pack_kv(np.ascontiguousarray(perm_kv)),
            "wq": Wq_p, "wk": Wk_p, "wv": Wvb,
        }
        if with_bq:
            m["bq"] = np.ascontiguousarray(bq.reshape(8, 128).T)
        if with_bk:
            m["bk"] = np.ascontiguousarray(bk.reshape(8, 128).T)
        if with_bv:
            m["bv"] = bv.astype(BF16).reshape(1, D)
        if with_mask:
            mt = mask[b, h * QL:(h + 1) * QL, :].T * np.float32(np.sqrt(D))
            if dist:
                mt = np.concatenate(
                    [mt[h * QL:(h + 1) * QL], mt[(1 - h) * QL:(2 - h) * QL]])
            m["maskt"] = np.ascontiguousarray(mt.astype(np.float32)).astype(BF16)
        in_maps.append(m)
    return flags, in_maps


def _ensure_axon_hooks_stub():
    # bass_utils imports antenv.axon_hooks when tracing is requested (even via
    # a stray BASS_TRACE env var); the module is absent on some images, so
    # register a no-op stub if needed.
    import sys, types
    try:
        import antenv.axon_hooks  # noqa: F401
    except ImportError:
        stub = types.ModuleType("antenv.axon_hooks")
        stub._hook = None
        stub.set_axon_ntff_profile_hook = (
            lambda h: setattr(stub, "_hook", h))
        stub.get_axon_ntff_profile_hook = lambda: stub._hook
        sys.modules["antenv.axon_hooks"] = stub
        try:
            import antenv
            antenv.axon_hooks = stub
        except ImportError:
            pass


def _run(inputs, trace=False, **kw):
    _ensure_axon_hooks_stub()
    from concourse import bass_utils
    dist = os.environ.get("KERNEL_DIST", "1") == "1"
    ps_bufs = int(os.environ.get("KERNEL_PSBUFS", "5"))
    flags, in_maps = _prep_in_maps(**inputs, dist=dist)
    nc = _get_nc(flags + (ps_bufs,))
    res = bass_utils.run_bass_kernel_spmd(
        nc, in_maps, core_ids=list(range(N_CORES)), trace=trace, **kw)
    out = np.empty((B, S, D), np.float32)
    for c in range(N_CORES):
        b, h = c // 2, c % 2
        out[b, h * QL:(h + 1) * QL, :] = np.asarray(
            res.results[c]["out"], dtype=np.float32)
    return out, res


def kernel(**inputs) -> np.ndarray:
    out, _ = _run(inputs, trace=False)
    return out


# revision 23
# speedup vs baseline: 1.0258x; 1.0258x over previous
"""Trainium2 Bass kernel: single-head attention (B=4, S=2048, D=1024) on 8 NeuronCores.

Sharding: data-parallel over (batch, query-half): core c handles batch c//2,
query rows [c%2*1024, (c%2+1)*1024), and (dist mode) computes the K-projection
only for its own kv half; the pair {2b, 2b+1} exchanges K^T via a 2-rank
AllGather overlapped with the Q-projection and the local-half attention.

Math per core (all matmuls bf16, fp32 PSUM accumulation):
  QT[dk,q]  = Wq(lhsT) . XqT(rhs)                 (+bq)
  KT[dk,s]  = Wk(lhsT) . XkvT(rhs)                (+bk)   [local half, AG for rest]
  sT[s,q]   = KT-tiles(lhsT) . QT(rhs)            scores^T
  eT[s,q]   = exp(sT / sqrt(dk) [+ maskT])        ScalarE, PSUM->SBUF bf16
  sums[1,q] = ones(lhsT) . eT(rhs)                softmax denominators
  HT[dm,q]  = Xkv-tiles(lhsT) . eT(rhs)           H = probs_unnorm @ Xkv
  out[q,dv] = HT-tiles(lhsT) . Wv(rhs)  (+bv)     (probs@Xkv)@Wv == probs@(Xkv@Wv)
  out      *= 1/sums  (per-partition scale on ScalarE, fused with PSUM->SBUF copy)

DMA design (from trace analysis):
- Each hw queue serves ~one dma_start at a time with multi-us turnaround, so
  aggregate startup bandwidth scales with in-flight dma_starts: keep payloads
  ~128-256KB and fan out across all ring slots of all 3 DMA-capable engines.
- A dma_start's wait condition blocks the ISSUING ENGINE's stream, so: the
  scalar engine (which runs every PSUM->SBUF activation) gets only a handful
  of pre-activation issues; collective readbacks (which wait on the AllGather)
  go on sync+gpsimd after everything else those engines must issue first.
- wq/wk are host-packed d_key-block-major, xqt/xkvt chunk-tile-major, and xkv
  (j, s)-tile-major, so every DMA has >=2KB contiguous segments and each PE
  accumulation group depends on the minimum payload (first matmul needs only
  ~0.5MB: wk block 0 + the leading 128-col xkvt chunk).

dist mode details: the s axis lives in LOCAL-relative order on each core
(m-tiles 0..7 = own kv half, 8..15 = partner's). The host feeds xkv/maskt
pre-permuted to match. The AllGather output is rank-ordered (identical layout
on both cores), so the partner block is recovered SPMD-uniformly via the exact
bf16-bit identity  remote = g0 XOR g1 XOR local  on uint32 views (own block
round-trips bit-identically through the collective).
"""

import os
import numpy as np
import ml_dtypes

B, S, D = 4, 2048, 1024
N_CORES = 8
QL = S // 2  # query rows per core (1024)
BF16 = ml_dtypes.bfloat16

_cache: dict = {}


def _kch(kvl):
    # K-projection chunk widths: narrow leading chunks shrink the
    # startup-critical DMA payload
    lead = int(os.environ.get("KERNEL_KCH", "256"))
    ch = [512] if lead == 512 else [lead, lead]
    while sum(ch) < kvl:
        ch.append(512)
    return ch


def _build(dist: bool, with_mask: bool, with_bq: bool, with_bk: bool,
           with_bv: bool, ps_bufs: int = 4):
    import concourse.bass as bass
    import concourse.mybir as mybir
    import concourse.tile as tile
    from concourse import bacc

    fp32 = mybir.dt.float32
    bf16 = mybir.dt.bfloat16
    uint32 = mybir.dt.uint32

    nc = bacc.Bacc("TRN2", target_bir_lowering=False, debug=False,
                   num_devices=N_CORES)

    KVL = QL if dist else S  # kv rows projected locally
    NT_D = D // 128    # 8 tiles along d_model / d_key
    NT_S = S // 128    # 16 tiles along s
    NT_L = NT_S // 2   # 8 (local-half s tiles in dist mode)
    NQ = QL // 512     # 2 query chunks of 512
    NV = D // 512      # 2 dv chunks of 512
    SCALE = 1.0 / float(np.sqrt(D))

    KCH = _kch(KVL)
    KOFF = [sum(KCH[:c]) for c in range(len(KCH))]
    LAST_C = len(KCH) - 1

    # all activations host-packed into SBUF tile layouts (fully-contiguous
    # DMA segments); wq/wk packed d_key-block-major
    xqt_d = nc.dram_tensor("xqt", (128, NT_D * QL), bf16, kind="ExternalInput")
    xkvt_d = nc.dram_tensor("xkvt", (128, NT_D * KVL), bf16,
                            kind="ExternalInput")
    xkv_d = nc.dram_tensor("xkv", (128, NT_D * S), bf16, kind="ExternalInput")
    wq_d = nc.dram_tensor("wq", (128, NT_D * D), bf16, kind="ExternalInput")
    wk_d = nc.dram_tensor("wk", (128, NT_D * D), bf16, kind="ExternalInput")
    wv_d = nc.dram_tensor("wv", (D, D), bf16, kind="ExternalInput")
    if with_bq:
        bq_d = nc.dram_tensor("bq", (128, 8), fp32, kind="ExternalInput")
    if with_bk:
        bk_d = nc.dram_tensor("bk", (128, 8), fp32, kind="ExternalInput")
    if with_bv:
        bv_d = nc.dram_tensor("bv", (1, D), bf16, kind="ExternalInput")
    if with_mask:
        maskt_d = nc.dram_tensor("maskt", (S, QL), bf16, kind="ExternalInput")
    out_d = nc.dram_tensor("out", (QL, D), bf16, kind="ExternalOutput")

    NSPLIT = 4  # K^T exchange split into NSPLIT AllGathers so the first
    # one triggers as soon as the first d_key rows land — the whole
    # collective+readback pipeline finishes ~30us earlier than a single AG
    IPG = NT_D // NSPLIT  # i-blocks per AG
    if dist:
        cc_ins = [nc.dram_tensor(f"cc_in{h}", (IPG * 128, QL), bf16)
                  for h in range(NSPLIT)]
        cc_outs = [nc.dram_tensor(f"cc_out{h}", (2 * IPG * 128, QL), bf16)
                   for h in range(NSPLIT)]
        groups = [[2 * g, 2 * g + 1] for g in range(4)]

    with tile.TileContext(nc) as tc:
        with (
            tc.tile_pool(name="cons", bufs=1) as cons,
            tc.tile_pool(name="wkb", bufs=1) as wkbp,
            tc.tile_pool(name="wqb", bufs=1) as wqbp,
            tc.tile_pool(name="big", bufs=3) as bigp,
            tc.tile_pool(name="kt", bufs=1) as ktp,
            tc.tile_pool(name="et", bufs=1) as etp,
            tc.tile_pool(name="xk", bufs=1) as xkp,
            tc.tile_pool(name="xq", bufs=2) as xqp,
            tc.tile_pool(name="xkv", bufs=4) as xkvp,
            tc.tile_pool(name="gch", bufs=2) as gchp,
            tc.tile_pool(name="outp", bufs=2) as outp,
            tc.tile_pool(name="mask", bufs=2) as maskp,
            tc.tile_pool(name="ps", bufs=ps_bufs,
                         space=bass.MemorySpace.PSUM) as psp,
            tc.tile_pool(name="pss", bufs=2, space=bass.MemorySpace.PSUM) as pssp,
            tc.tile_pool(name="pst", bufs=1, space=bass.MemorySpace.PSUM) as pstp,
        ):
            # ---- startup-critical DMA wave ----
            wk_blks = [wkbp.tile([128, D], bf16, tag=f"wkb{i}",
                                 name=f"wkb{i}")
                       for i in range(NT_D)]
            xk_chs = [xkp.tile([128, NT_D, KCH[c]], bf16,
                               tag=f"xk{c}", name=f"xk{c}")
                      for c in range(len(KCH))]

            def ld_wk(eng, i):   # one i-block: 256KB
                eng.dma_start(wk_blks[i][:],
                              wk_d.ap()[:, i * D:(i + 1) * D])

            def ld_xk(eng, c, part, nparts):  # 1/nparts of chunk c
                w = KCH[c]
                jn = NT_D // nparts
                base = NT_D * KOFF[c]
                eng.dma_start(
                    xk_chs[c][:, part * jn:(part + 1) * jn, :],
                    xkvt_d.ap()[:, base + part * jn * w:
                                base + (part + 1) * jn * w])

            sy, sc, gp = nc.sync, nc.scalar, nc.gpsimd
            # (engine, payload) in consumer-priority order. ~256-512KB per
            # dma_start: few enough issue ops (~0.65us each) that the whole
            # 4MB critical set is in flight by ~12us, big enough to keep the
            # queue ring slots saturated. scalar gets only 2 issues so its
            # first K activation isn't delayed.
            ld_wk(sy, 0); ld_xk(sc, 0, 0, 2); ld_xk(gp, 0, 1, 2)
            ld_wk(sy, 1); ld_wk(sc, 2); ld_wk(gp, 3)
            ld_xk(sy, 1, 0, 2); ld_xk(gp, 1, 1, 2)
            ld_wk(sy, 4); ld_wk(gp, 5)
            ld_xk(sy, 2, 0, 2); ld_xk(gp, 2, 1, 2)
            ld_wk(sy, 6); ld_wk(gp, 7)
            for c in range(3, len(KCH)):
                ld_xk(sy, c, 0, 2); ld_xk(gp, c, 1, 2)

            # ---- Q-stage loads (needed ~28us later): sync + gpsimd ----
            wq_blks = [wqbp.tile([128, D], bf16, tag=f"wqb{i}",
                                 name=f"wqb{i}")
                       for i in range(NT_D)]
            xq_chs = [xqp.tile([128, NT_D, 512], bf16, tag="xq",
                               name=f"xq{n}")
                      for n in range(NQ)]

            def ld_xq(eng, n, part):
                eng.dma_start(
                    xq_chs[n][:, part * 4:(part + 1) * 4, :],
                    xqt_d.ap()[:, n * NT_D * 512 + part * 2048:
                               n * NT_D * 512 + (part + 1) * 2048])

            ld_xq(sy, 0, 0); ld_xq(gp, 0, 1)
            for i in range(NT_D):
                (sy if i % 2 == 0 else gp).dma_start(
                    wq_blks[i][:], wq_d.ap()[:, i * D:(i + 1) * D])
            ld_xq(sy, 1, 0); ld_xq(gp, 1, 1)

            # wv on gpsimd BEFORE the collective trigger (which blocks the
            # gpsimd stream until the cc_in stores land)
            wv_sb = bigp.tile([128, NT_D, D], bf16, tag="big", name="wv_sb")
            for h in range(4):
                nc.gpsimd.dma_start(
                    wv_sb[:, 2 * h:2 * h + 2, :],
                    wv_d.ap()[2 * h * 128:(2 * h + 2) * 128, :]
                    .rearrange("(j p) d -> p j d", p=128))

            # ---- constants / biases (tiny, off the critical path) ----
            ones_col = cons.tile([128, 1], fp32, tag="ones_col")
            nc.gpsimd.memset(ones_col[:], 1.0)
            ident1 = cons.tile([1, 1], fp32, tag="ident1")
            nc.gpsimd.memset(ident1[:], 1.0)
            if with_bv:
                bv_sb = cons.tile([1, D], bf16, tag="bv")
                nc.sync.dma_start(bv_sb[:], bv_d.ap()[:])
            if with_bq:
                bq_sb = cons.tile([128, 8], fp32, tag="bq")
                nc.sync.dma_start(bq_sb[:], bq_d.ap()[:])
            if with_bk:
                bk_sb = cons.tile([128, 8], fp32, tag="bk")
                nc.sync.dma_start(bk_sb[:], bk_d.ap()[:])

            kt_sb = ktp.tile([128, NT_D, KVL], bf16, tag="kt")
            if dist:
                ktr_sb = ktp.tile([128, NT_D, QL], bf16, tag="ktr")
            et_sb = etp.tile([128, NT_S, QL], bf16, tag="et")

            # xkv prefetch helper (gpsimd): packed (j, s)-tile-major layout,
            # one contiguous 256KB dma_start per (pass, j)
            xkv_chunks = {}

            def prefetch_xkv(ms, js):
                for j in js:
                    xkv_ch = xkvp.tile([128, len(ms), 128], bf16, tag="xkv",
                                       name=f"xkv{ms[0]}_{j}")
                    nc.gpsimd.dma_start(
                        xkv_ch[:],
                        xkv_d.ap()[:, j * NT_S * 128 + ms[0] * 128:
                                   j * NT_S * 128 + (ms[-1] + 1) * 128])
                    xkv_chunks[(j, ms[0])] = xkv_ch

            # first-half xkv for stage 3a: issue the first 4 (= pool depth)
            # before the collective occupies the gpsimd stream
            local_ms = list(range(NT_L)) if dist else list(range(NT_S))
            prefetch_xkv(local_ms, range(4))

            # ---- stage 1b: KT (local half in dist mode) -> kt_sb ----
            # split-outer/chunk-outer: the first d_key row-pairs finish first
            # (feeding AllGather #1 early) and the first groups need only
            # chunk 0; cc_in row-stores go out inline per row block.
            for half in range(NSPLIT):
                ilist = range(half * IPG, (half + 1) * IPG)
                for c in range(len(KCH)):
                    w, off = KCH[c], KOFF[c]
                    for i in ilist:
                        ps = psp.tile([128, 512], fp32, tag="ps")
                        for j in range(NT_D):
                            nc.tensor.matmul(
                                ps[:, :w],
                                wk_blks[i][:, j * 128:(j + 1) * 128],
                                xk_chs[c][:, j, :],
                                start=(j == 0), stop=(j == NT_D - 1))
                        if with_bk:
                            nc.scalar.activation(
                                kt_sb[:, i, off:off + w], ps[:, :w],
                                mybir.ActivationFunctionType.Identity,
                                bias=bk_sb[:, i:i + 1])
                        else:
                            nc.scalar.activation(
                                kt_sb[:, i, off:off + w], ps[:, :w],
                                mybir.ActivationFunctionType.Copy)
                        if dist and c == LAST_C:
                            (nc.sync if i % 2 == 0 else nc.scalar).dma_start(
                                cc_ins[half].ap()[(i % IPG) * 128:
                                                  (i % IPG + 1) * 128, :],
                                kt_sb[:, i, 0:QL])
                if dist:
                    nc.gpsimd.collective_compute(
                        "AllGather", mybir.AluOpType.bypass,
                        replica_groups=groups,
                        ins=[cc_ins[half].ap()[:].opt()],
                        outs=[cc_outs[half].ap()[:].opt()],
                    )

            # ---- stage 1a: QT -> qt_sb ----
            qt_sb = bigp.tile([128, NT_D, D], bf16, tag="big", name="qt_sb")
            for n in range(NQ):
                for i in range(NT_D):
                    ps = psp.tile([128, 512], fp32, tag="ps")
                    for j in range(NT_D):
                        nc.tensor.matmul(
                            ps[:], wq_blks[i][:, j * 128:(j + 1) * 128],
                            xq_chs[n][:, j, :],
                            start=(j == 0), stop=(j == NT_D - 1))
                    if with_bq:
                        nc.scalar.activation(
                            qt_sb[:, i, n * 512:(n + 1) * 512], ps[:],
                            mybir.ActivationFunctionType.Identity,
                            bias=bq_sb[:, i:i + 1])
                    else:
                        nc.scalar.activation(
                            qt_sb[:, i, n * 512:(n + 1) * 512], ps[:],
                            mybir.ActivationFunctionType.Copy)

            if dist:
                # read the gathered pair back, recover the partner's block via
                # remote = g0 ^ g1 ^ local (exact bf16 bit identity) -> ktr_sb.
                # sync + gpsimd (both are past their other issue duties; the
                # collective-wait embedded in these DMAs blocks the engine,
                # so they must never sit on the scalar stream)
                for i in range(NT_D):
                    cc_view = cc_outs[i // IPG].ap().rearrange(
                        "(b r) f -> r b f", b=2)
                    r0 = (i % IPG) * 128
                    g_ch = gchp.tile([128, 2, QL], bf16, tag="gch",
                                     name=f"gch{i}")
                    eng = nc.sync if i % 2 == 0 else nc.gpsimd
                    eng.dma_start(
                        g_ch[:, 0, :], cc_view[r0:r0 + 128, 0, :])
                    eng.dma_start(
                        g_ch[:, 1, :], cc_view[r0:r0 + 128, 1, :])
                    nc.vector.tensor_tensor(
                        g_ch[:, 0, :].bitcast(uint32),
                        g_ch[:, 0, :].bitcast(uint32),
                        g_ch[:, 1, :].bitcast(uint32),
                        mybir.AluOpType.bitwise_xor)
                    nc.vector.tensor_tensor(
                        ktr_sb[:, i, :].bitcast(uint32),
                        g_ch[:, 0, :].bitcast(uint32),
                        kt_sb[:, i, :].bitcast(uint32),
                        mybir.AluOpType.bitwise_xor)

            # ---- stage 2: scores^T + exp ----
            def score_group(m, n):
                kt, mm = (ktr_sb, m - NT_L) if (dist and m >= NT_L) else (kt_sb, m)
                ps = psp.tile([128, 512], fp32, tag="ps")
                for i in range(NT_D):
                    nc.tensor.matmul(
                        ps[:], kt[:, i, mm * 128:(mm + 1) * 128],
                        qt_sb[:, i, n * 512:(n + 1) * 512],
                        start=(i == 0), stop=(i == NT_D - 1))
                if with_mask:
                    mk = maskp.tile([128, 512], bf16, tag="mask")
                    nc.sync.dma_start(
                        mk[:], maskt_d.ap()[m * 128:(m + 1) * 128,
                                            n * 512:(n + 1) * 512])
                    nc.vector.tensor_tensor(
                        ps[:], ps[:], mk[:], mybir.AluOpType.add)
                nc.scalar.activation(
                    et_sb[:, m, n * 512:(n + 1) * 512], ps[:],
                    mybir.ActivationFunctionType.Exp, scale=SCALE)

            first_ms = range(NT_L) if dist else range(NT_S)
            for n in range(NQ):
                for m in first_ms:
                    score_group(m, n)

            # softmax denominators: accumulate expT tiles on the DVE (PE
            # has no slack; DVE has plenty). In-place fp32 chain.
            sacc = cons.tile([128, QL], fp32, tag="sacc")
            first_l = list(first_ms)
            nc.vector.tensor_tensor(
                sacc[:], et_sb[:, first_l[0], :], et_sb[:, first_l[1], :],
                mybir.AluOpType.add)
            for m in first_l[2:]:
                nc.vector.tensor_tensor(
                    sacc[:], sacc[:], et_sb[:, m, :], mybir.AluOpType.add)

            # ---- stage 3a: HT over available s-tiles ----
            ht_sb = bigp.tile([128, NT_D, D], bf16, tag="big", name="ht_sb")

            def ht_groups(ms, merge, skip_prefetch=0):
                prefetch_xkv(ms, range(skip_prefetch, NT_D))
                for j in range(NT_D):
                    xkv_ch = xkv_chunks[(j, ms[0])]
                    for n in range(NQ):
                        ps = psp.tile([128, 512], fp32, tag="ps")
                        for k2, m in enumerate(ms):
                            nc.tensor.matmul(
                                ps[:], xkv_ch[:, k2, :],
                                et_sb[:, m, n * 512:(n + 1) * 512],
                                start=(k2 == 0), stop=(k2 == len(ms) - 1))
                        dst = ht_sb[:, j, n * 512:(n + 1) * 512]
                        if merge:
                            nc.vector.tensor_tensor(
                                dst, ps[:], dst, mybir.AluOpType.add)
                        else:
                            nc.scalar.activation(
                                dst, ps[:],
                                mybir.ActivationFunctionType.Copy)

            if dist:
                ht_groups(local_ms, merge=False, skip_prefetch=4)
                # m-outer: exp(m) completes both q-chunks back-to-back so the
                # DVE sums chain below never lags the PE
                for m in range(NT_L, NT_S):
                    for n in range(NQ):
                        score_group(m, n)
                for m in range(NT_L, NT_S):
                    nc.vector.tensor_tensor(
                        sacc[:], sacc[:], et_sb[:, m, :], mybir.AluOpType.add)
                ht_groups(list(range(NT_L, NT_S)), merge=True)
            else:
                ht_groups(local_ms, merge=False, skip_prefetch=4)

            # sums[1, q]: single fp32 ones-matmul per q-chunk over sacc.
            # Allocated here; EMITTED inside stage 4 after the first output
            # group so the PE chews useful matmuls while the DVE chain ends.
            sums_sb = cons.tile([1, QL], fp32, tag="sums")
            pst = pstp.tile([128, 8], fp32, tag="pst")
            recip_sb = cons.tile([128, 8], fp32, tag="recip")
            if with_bv:
                sums_bf = cons.tile([1, QL], bf16, tag="sums_bf")

            def emit_sums():
                for n in range(NQ):
                    pss = pssp.tile([1, 512], fp32, tag="pss")
                    nc.tensor.matmul(
                        pss[:], ones_col[:], sacc[:, n * 512:(n + 1) * 512],
                        start=True, stop=True)
                    nc.scalar.activation(
                        sums_sb[:, n * 512:(n + 1) * 512], pss[:],
                        mybir.ActivationFunctionType.Copy)
                for p in range(8):
                    nc.tensor.transpose(
                        pst[:, p:p + 1], sums_sb[:, p * 128:(p + 1) * 128],
                        ident1[:])
                nc.vector.reciprocal(recip_sb[:], pst[:])
                if with_bv:
                    # out accumulates UNNORMALIZED; bias enters as sums[q]*bv
                    # so the final 1/sums scale leaves exactly +bv
                    nc.scalar.activation(sums_bf[:], sums_sb[:],
                                         mybir.ActivationFunctionType.Copy)

            # ---- stage 4: out = HT^T . Wv (+bv), normalized, bf16 out ----
            # p=0: matmuls first, then the sums block (PE stays busy while
            # the DVE chain finishes), then the p=0 normalization.
            for p in range(8):
                out_sb = outp.tile([128, D], bf16, tag="outsb")
                group_ps = []
                for n2 in range(NV):
                    ps = psp.tile([128, 512], fp32, tag="ps")
                    for j in range(NT_D):
                        nc.tensor.matmul(
                            ps[:], ht_sb[:, j, p * 128:(p + 1) * 128],
                            wv_sb[:, j, n2 * 512:(n2 + 1) * 512],
                            start=(j == 0),
                            stop=(j == NT_D - 1 and not with_bv))
                    group_ps.append(ps)
                if p == 0:
                    emit_sums()
                for n2, ps in enumerate(group_ps):
                    if with_bv:
                        nc.tensor.matmul(
                            ps[:], sums_bf[:, p * 128:(p + 1) * 128],
                            bv_sb[:, n2 * 512:(n2 + 1) * 512],
                            start=False, stop=True)
                    nc.scalar.activation(
                        out_sb[:, n2 * 512:(n2 + 1) * 512], ps[:],
                        mybir.ActivationFunctionType.Copy,
                        scale=recip_sb[:, p:p + 1])
                    if p == 7 and n2 == NV - 1:
                        # final chunk: split + both queues for a short drain
                        for hh in range(2):
                            lo = n2 * 512 + hh * 256
                            (nc.sync if hh == 0 else nc.scalar).dma_start(
                                out_d.ap()[p * 128:(p + 1) * 128, lo:lo + 256],
                                out_sb[:, lo:lo + 256])
                    else:
                        (nc.sync if (p + n2) % 2 == 0 else
                         nc.scalar).dma_start(
                            out_d.ap()[p * 128:(p + 1) * 128,
                                       n2 * 512:(n2 + 1) * 512],
                            out_sb[:, n2 * 512:(n2 + 1) * 512])

    nc.compile()
    return nc


def _get_nc(flags):
    if flags not in _cache:
        _cache[flags] = _build(*flags)
    return _cache[flags]


def _flags_of(inputs, dist=True):
    return _prep_in_maps(**inputs, dist=dist)[0]


def _pack_w(W):
    # [j*128+p, i*128+c] -> [p, i*1024 + j*128 + c]
    return np.ascontiguousarray(
        W.reshape(8, 128, 8, 128).transpose(1, 2, 0, 3).reshape(128, 8192))


def _pack_x(xt, widths):
    # xt [D, L] (row-major) -> [128, sum_c 8*w_c]: for each col-chunk c,
    # block[p, j*w + t] = xt[j*128+p, off+t]  (SBUF tile layout, so each
    # chunk is one fully-contiguous DMA)
    blocks = []
    off = 0
    for w in widths:
        blk = xt[:, off:off + w].reshape(8, 128, w).transpose(1, 0, 2)
        blocks.append(blk.reshape(128, 8 * w))
        off += w
    return np.ascontiguousarray(np.concatenate(blocks, axis=1))


def _pack_kv(xkv):
    # xkv [S, D] -> [128, j*S + m*128 + t] with xkv[m*128+p, j*128+t]:
    # per (s-pass, j) loads are fully contiguous
    nts = xkv.shape[0] // 128
    return np.ascontiguousarray(
        xkv.reshape(nts, 128, 8, 128).transpose(1, 2, 0, 3)
        .reshape(128, nts * 1024))


def _prep_in_maps(query_input, keyvalue_input, mask, Wq, bq, Wk, bk, Wv, bv,
                  dist=True):
    qi = np.asarray(query_input, np.float32)
    kv = np.asarray(keyvalue_input, np.float32)
    mask = np.asarray(mask, np.float32)
    Wqb = np.asarray(Wq, np.float32).astype(BF16)
    Wkb = np.asarray(Wk, np.float32).astype(BF16)
    Wvb = np.asarray(Wv, np.float32).astype(BF16)
    bq = np.asarray(bq, np.float32)
    bk = np.asarray(bk, np.float32)
    bv = np.asarray(bv, np.float32)

    with_mask = bool(np.any(mask != 0.0))
    with_bq = bool(np.any(bq != 0.0))
    with_bk = bool(np.any(bk != 0.0))
    with_bv = bool(np.any(bv != 0.0))
    flags = (dist, with_mask, with_bq, with_bk, with_bv)

    Wq_p = _pack_w(Wqb)
    Wk_p = _pack_w(Wkb)

    in_maps = []
    for c in range(N_CORES):
        b, h = c // 2, c % 2
        xq = qi[b, h * QL:(h + 1) * QL, :].astype(BF16)       # [QL, D]
        xkv = kv[b].astype(BF16)                               # [S, D]
        if dist:
            xkvt = np.ascontiguousarray(xkv[h * QL:(h + 1) * QL, :].T)
            perm_kv = np.concatenate(
                [xkv[h * QL:(h + 1) * QL], xkv[(1 - h) * QL:(2 - h) * QL]])
        else:
            xkvt = np.ascontiguousarray(xkv.T)
            perm_kv = xkv
        m = {
            "xqt": _pack_x(np.ascontiguousarray(xq.T), [512, 512]),
            "xkvt": _pack_x(xkvt, _kch(xkvt.shape[1])),
            "xkv": _pack_kv(np.ascontiguousarray(perm_kv)),
            "wq": Wq_p, "wk": Wk_p, "wv": Wvb,
        }
        if with_bq:
            m["bq"] = np.ascontiguousarray(bq.reshape(8, 128).T)
        if with_bk:
            m["bk"] = np.ascontiguousarray(bk.reshape(8, 128).T)
        if with_bv:
            m["bv"] = bv.astype(BF16).reshape(1, D)
        if with_mask:
            mt = mask[b, h * QL:(h + 1) * QL, :].T * np.float32(np.sqrt(D))
            if dist:
                mt = np.concatenate(
                    [mt[h * QL:(h + 1) * QL], mt[(1 - h) * QL:(2 - h) * QL]])
            m["maskt"] = np.ascontiguousarray(mt.astype(np.float32)).astype(BF16)
        in_maps.append(m)
    return flags, in_maps


def _ensure_axon_hooks_stub():
    # bass_utils imports antenv.axon_hooks when tracing is requested (even via
    # a stray BASS_TRACE env var); the module is absent on some images, so
    # register a no-op stub if needed.
    import sys, types
    try:
        import antenv.axon_hooks  # noqa: F401
    except ImportError:
        stub = types.ModuleType("antenv.axon_hooks")
        stub._hook = None
        stub.set_axon_ntff_profile_hook = (
            lambda h: setattr(stub, "_hook", h))
        stub.get_axon_ntff_profile_hook = lambda: stub._hook
        sys.modules["antenv.axon_hooks"] = stub
        try:
            import antenv
            antenv.axon_hooks = stub
        except ImportError:
            pass


def _run(inputs, trace=False, **kw):
    _ensure_axon_hooks_stub()
    from concourse import bass_utils
    dist = os.environ.get("KERNEL_DIST", "1") == "1"
    ps_bufs = int(os.environ.get("KERNEL_PSBUFS", "5"))
    flags, in_maps = _prep_in_maps(**inputs, dist=dist)
    nc = _get_nc(flags + (ps_bufs,))
    res = bass_utils.run_bass_kernel_spmd(
        nc, in_maps, core_ids=list(range(N_CORES)), trace=trace, **kw)
    out = np.empty((B, S, D), np.float32)
    for c in range(N_CORES):
        b, h = c // 2, c % 2
        out[b, h * QL:(h + 1) * QL, :] = np.asarray(
            res.results[c]["out"], dtype=np.float32)
    return out, res


def kernel(**inputs) -> np.ndarray:
    out, _ = _run(inputs, trace=False)
    return out


# revision 26
# speedup vs baseline: 1.0346x; 1.0086x over previous
"""Trainium2 Bass kernel: single-head attention (B=4, S=2048, D=1024) on 8 NeuronCores.

Sharding: data-parallel over (batch, query-half): core c handles batch c//2,
query rows [c%2*1024, (c%2+1)*1024), and (dist mode) computes the K-projection
only for its own kv half; the pair {2b, 2b+1} exchanges K^T via a 2-rank
AllGather overlapped with the Q-projection and the local-half attention.

Math per core (all matmuls bf16, fp32 PSUM accumulation):
  QT[dk,q]  = Wq(lhsT) . XqT(rhs)                 (+bq)
  KT[dk,s]  = Wk(lhsT) . XkvT(rhs)                (+bk)   [local half, AG for rest]
  sT[s,q]   = KT-tiles(lhsT) . QT(rhs)            scores^T
  eT[s,q]   = exp(sT / sqrt(dk) [+ maskT])        ScalarE, PSUM->SBUF bf16
  sums[1,q] = ones(lhsT) . eT(rhs)                softmax denominators
  HT[dm,q]  = Xkv-tiles(lhsT) . eT(rhs)           H = probs_unnorm @ Xkv
  out[q,dv] = HT-tiles(lhsT) . Wv(rhs)  (+bv)     (probs@Xkv)@Wv == probs@(Xkv@Wv)
  out      *= 1/sums  (per-partition scale on ScalarE, fused with PSUM->SBUF copy)

DMA design (from trace analysis):
- Each hw queue serves ~one dma_start at a time with multi-us turnaround, so
  aggregate startup bandwidth scales with in-flight dma_starts: keep payloads
  ~128-256KB and fan out across all ring slots of all 3 DMA-capable engines.
- A dma_start's wait condition blocks the ISSUING ENGINE's stream, so: the
  scalar engine (which runs every PSUM->SBUF activation) gets only a handful
  of pre-activation issues; collective readbacks (which wait on the AllGather)
  go on sync+gpsimd after everything else those engines must issue first.
- wq/wk are host-packed d_key-block-major, xqt/xkvt chunk-tile-major, and xkv
  (j, s)-tile-major, so every DMA has >=2KB contiguous segments and each PE
  accumulation group depends on the minimum payload (first matmul needs only
  ~0.5MB: wk block 0 + the leading 128-col xkvt chunk).

dist mode details: the s axis lives in LOCAL-relative order on each core
(m-tiles 0..7 = own kv half, 8..15 = partner's). The host feeds xkv/maskt
pre-permuted to match. The AllGather output is rank-ordered (identical layout
on both cores), so the partner block is recovered SPMD-uniformly via the exact
bf16-bit identity  remote = g0 XOR g1 XOR local  on uint32 views (own block
round-trips bit-identically through the collective).
"""

import os
import numpy as np
import ml_dtypes

B, S, D = 4, 2048, 1024
N_CORES = 8
QL = S // 2  # query rows per core (1024)
BF16 = ml_dtypes.bfloat16

_cache: dict = {}


def _kch(kvl):
    # K-projection chunk widths: narrow leading chunks shrink the
    # startup-critical DMA payload
    lead = int(os.environ.get("KERNEL_KCH", "256"))
    ch = [512] if lead == 512 else [lead, lead]
    while sum(ch) < kvl:
        ch.append(512)
    return ch


def _build(dist: bool, with_mask: bool, with_bq: bool, with_bk: bool,
           with_bv: bool, ps_bufs: int = 4):
    import concourse.bass as bass
    import concourse.mybir as mybir
    import concourse.tile as tile
    from concourse import bacc

    fp32 = mybir.dt.float32
    bf16 = mybir.dt.bfloat16
    uint32 = mybir.dt.uint32

    nc = bacc.Bacc("TRN2", target_bir_lowering=False, debug=False,
                   num_devices=N_CORES)

    KVL = QL if dist else S  # kv rows projected locally
    NT_D = D // 128    # 8 tiles along d_model / d_key
    NT_S = S // 128    # 16 tiles along s
    NT_L = NT_S // 2   # 8 (local-half s tiles in dist mode)
    NQ = QL // 512     # 2 query chunks of 512
    NV = D // 512      # 2 dv chunks of 512
    SCALE = 1.0 / float(np.sqrt(D))

    KCH = _kch(KVL)
    KOFF = [sum(KCH[:c]) for c in range(len(KCH))]
    LAST_C = len(KCH) - 1

    # all activations host-packed into SBUF tile layouts (fully-contiguous
    # DMA segments); wq/wk packed d_key-block-major
    xqt_d = nc.dram_tensor("xqt", (128, NT_D * QL), bf16, kind="ExternalInput")
    xkvt_d = nc.dram_tensor("xkvt", (128, NT_D * KVL), bf16,
                            kind="ExternalInput")
    xkv_d = nc.dram_tensor("xkv", (128, NT_D * S), bf16, kind="ExternalInput")
    wq_d = nc.dram_tensor("wq", (128, NT_D * D), bf16, kind="ExternalInput")
    wk_d = nc.dram_tensor("wk", (128, NT_D * D), bf16, kind="ExternalInput")
    wv_d = nc.dram_tensor("wv", (D, D), bf16, kind="ExternalInput")
    if with_bq:
        bq_d = nc.dram_tensor("bq", (128, 8), fp32, kind="ExternalInput")
    if with_bk:
        bk_d = nc.dram_tensor("bk", (128, 8), fp32, kind="ExternalInput")
    if with_bv:
        bv_d = nc.dram_tensor("bv", (1, D), bf16, kind="ExternalInput")
    if with_mask:
        maskt_d = nc.dram_tensor("maskt", (S, QL), bf16, kind="ExternalInput")
    out_d = nc.dram_tensor("out", (QL, D), bf16, kind="ExternalOutput")

    NSPLIT = 4  # K^T exchange split into NSPLIT AllGathers so the first
    # one triggers as soon as the first d_key rows land — the whole
    # collective+readback pipeline finishes ~30us earlier than a single AG
    IPG = NT_D // NSPLIT  # i-blocks per AG
    if dist:
        cc_ins = [nc.dram_tensor(f"cc_in{h}", (IPG * 128, QL), bf16)
                  for h in range(NSPLIT)]
        cc_outs = [nc.dram_tensor(f"cc_out{h}", (2 * IPG * 128, QL), bf16)
                   for h in range(NSPLIT)]
        groups = [[2 * g, 2 * g + 1] for g in range(4)]

    with tile.TileContext(nc) as tc:
        with (
            tc.tile_pool(name="cons", bufs=1) as cons,
            tc.tile_pool(name="wkb", bufs=1) as wkbp,
            tc.tile_pool(name="wqb", bufs=1) as wqbp,
            tc.tile_pool(name="big", bufs=3) as bigp,
            tc.tile_pool(name="kt", bufs=1) as ktp,
            tc.tile_pool(name="et", bufs=1) as etp,
            tc.tile_pool(name="xk", bufs=1) as xkp,
            tc.tile_pool(name="xq", bufs=2) as xqp,
            tc.tile_pool(name="xkv", bufs=4) as xkvp,
            tc.tile_pool(name="gch", bufs=2) as gchp,
            tc.tile_pool(name="outp", bufs=2) as outp,
            tc.tile_pool(name="mask", bufs=2) as maskp,
            tc.tile_pool(name="ps", bufs=ps_bufs,
                         space=bass.MemorySpace.PSUM) as psp,
            tc.tile_pool(name="pss", bufs=2, space=bass.MemorySpace.PSUM) as pssp,
            tc.tile_pool(name="pst", bufs=1, space=bass.MemorySpace.PSUM) as pstp,
        ):
            # ---- startup-critical DMA wave ----
            wk_blks = [wkbp.tile([128, D], bf16, tag=f"wkb{i}",
                                 name=f"wkb{i}")
                       for i in range(NT_D)]
            xk_chs = [xkp.tile([128, NT_D, KCH[c]], bf16,
                               tag=f"xk{c}", name=f"xk{c}")
                      for c in range(len(KCH))]

            def ld_wk(eng, i):   # one i-block: 256KB
                eng.dma_start(wk_blks[i][:],
                              wk_d.ap()[:, i * D:(i + 1) * D])

            def ld_xk(eng, c, part, nparts):  # 1/nparts of chunk c
                w = KCH[c]
                jn = NT_D // nparts
                base = NT_D * KOFF[c]
                eng.dma_start(
                    xk_chs[c][:, part * jn:(part + 1) * jn, :],
                    xkvt_d.ap()[:, base + part * jn * w:
                                base + (part + 1) * jn * w])

            sy, sc, gp = nc.sync, nc.scalar, nc.gpsimd
            # (engine, payload) in consumer-priority order. ~256-512KB per
            # dma_start: few enough issue ops (~0.65us each) that the whole
            # 4MB critical set is in flight by ~12us, big enough to keep the
            # queue ring slots saturated. scalar gets only 2 issues so its
            # first K activation isn't delayed.
            # the very first pieces are halved again: cold rings run at
            # ~100GB/s, so 128KB pieces put the first matmul ~2us earlier
            sy.dma_start(wk_blks[0][:, 0:512], wk_d.ap()[:, 0:512])
            sc.dma_start(wk_blks[0][:, 512:D], wk_d.ap()[:, 512:D])
            ld_xk(gp, 0, 0, 4); ld_xk(sy, 0, 1, 4)
            ld_xk(sc, 0, 2, 4); ld_xk(gp, 0, 3, 4)
            ld_wk(sy, 1); ld_wk(sc, 2); ld_wk(gp, 3)
            ld_xk(sy, 1, 0, 2); ld_xk(gp, 1, 1, 2)
            ld_wk(sy, 4); ld_wk(gp, 5)
            ld_xk(sy, 2, 0, 2); ld_xk(gp, 2, 1, 2)
            ld_wk(sy, 6); ld_wk(gp, 7)
            for c in range(3, len(KCH)):
                ld_xk(sy, c, 0, 2); ld_xk(gp, c, 1, 2)

            # ---- Q-stage loads (needed ~28us later): sync + gpsimd ----
            wq_blks = [wqbp.tile([128, D], bf16, tag=f"wqb{i}",
                                 name=f"wqb{i}")
                       for i in range(NT_D)]
            xq_chs = [xqp.tile([128, NT_D, 512], bf16, tag="xq",
                               name=f"xq{n}")
                      for n in range(NQ)]

            def ld_xq(eng, n, part):
                eng.dma_start(
                    xq_chs[n][:, part * 4:(part + 1) * 4, :],
                    xqt_d.ap()[:, n * NT_D * 512 + part * 2048:
                               n * NT_D * 512 + (part + 1) * 2048])

            ld_xq(sy, 0, 0); ld_xq(gp, 0, 1)
            for i in range(NT_D):
                (sy if i % 2 == 0 else gp).dma_start(
                    wq_blks[i][:], wq_d.ap()[:, i * D:(i + 1) * D])
            ld_xq(sy, 1, 0); ld_xq(gp, 1, 1)

            # wv on gpsimd BEFORE the collective trigger (which blocks the
            # gpsimd stream until the cc_in stores land)
            wv_sb = bigp.tile([128, NT_D, D], bf16, tag="big", name="wv_sb")
            for h in range(4):
                nc.gpsimd.dma_start(
                    wv_sb[:, 2 * h:2 * h + 2, :],
                    wv_d.ap()[2 * h * 128:(2 * h + 2) * 128, :]
                    .rearrange("(j p) d -> p j d", p=128))

            # ---- constants / biases (tiny, off the critical path) ----
            ones_col = cons.tile([128, 1], fp32, tag="ones_col")
            nc.gpsimd.memset(ones_col[:], 1.0)
            ident1 = cons.tile([1, 1], fp32, tag="ident1")
            nc.gpsimd.memset(ident1[:], 1.0)
            if with_bv:
                bv_sb = cons.tile([1, D], bf16, tag="bv")
                nc.sync.dma_start(bv_sb[:], bv_d.ap()[:])
            if with_bq:
                bq_sb = cons.tile([128, 8], fp32, tag="bq")
                nc.sync.dma_start(bq_sb[:], bq_d.ap()[:])
            if with_bk:
                bk_sb = cons.tile([128, 8], fp32, tag="bk")
                nc.sync.dma_start(bk_sb[:], bk_d.ap()[:])

            kt_sb = ktp.tile([128, NT_D, KVL], bf16, tag="kt")
            if dist:
                ktr_sb = ktp.tile([128, NT_D, QL], bf16, tag="ktr")
            et_sb = etp.tile([128, NT_S, QL], bf16, tag="et")

            # xkv prefetch helper (gpsimd): packed (j, s)-tile-major layout,
            # one contiguous 256KB dma_start per (pass, j)
            xkv_chunks = {}

            def prefetch_xkv(ms, js):
                for j in js:
                    xkv_ch = xkvp.tile([128, len(ms), 128], bf16, tag="xkv",
                                       name=f"xkv{ms[0]}_{j}")
                    nc.gpsimd.dma_start(
                        xkv_ch[:],
                        xkv_d.ap()[:, j * NT_S * 128 + ms[0] * 128:
                                   j * NT_S * 128 + (ms[-1] + 1) * 128])
                    xkv_chunks[(j, ms[0])] = xkv_ch

            # first-half xkv for stage 3a: issue the first 4 (= pool depth)
            # before the collective occupies the gpsimd stream
            local_ms = list(range(NT_L)) if dist else list(range(NT_S))
            prefetch_xkv(local_ms, range(4))

            # ---- stage 1b: KT (local half in dist mode) -> kt_sb ----
            # split-outer/chunk-outer: the first d_key row-pairs finish first
            # (feeding AllGather #1 early) and the first groups need only
            # chunk 0; cc_in row-stores go out inline per row block.
            for half in range(NSPLIT):
                ilist = range(half * IPG, (half + 1) * IPG)
                for c in range(len(KCH)):
                    w, off = KCH[c], KOFF[c]
                    for i in ilist:
                        ps = psp.tile([128, 512], fp32, tag="ps")
                        for j in range(NT_D):
                            nc.tensor.matmul(
                                ps[:, :w],
                                wk_blks[i][:, j * 128:(j + 1) * 128],
                                xk_chs[c][:, j, :],
                                start=(j == 0), stop=(j == NT_D - 1))
                        if with_bk:
                            nc.scalar.activation(
                                kt_sb[:, i, off:off + w], ps[:, :w],
                                mybir.ActivationFunctionType.Identity,
                                bias=bk_sb[:, i:i + 1])
                        else:
                            nc.scalar.activation(
                                kt_sb[:, i, off:off + w], ps[:, :w],
                                mybir.ActivationFunctionType.Copy)
                        if dist and c == LAST_C:
                            (nc.sync if i % 2 == 0 else nc.scalar).dma_start(
                                cc_ins[half].ap()[(i % IPG) * 128:
                                                  (i % IPG + 1) * 128, :],
                                kt_sb[:, i, 0:QL])
                if dist:
                    nc.gpsimd.collective_compute(
                        "AllGather", mybir.AluOpType.bypass,
                        replica_groups=groups,
                        ins=[cc_ins[half].ap()[:].opt()],
                        outs=[cc_outs[half].ap()[:].opt()],
                    )

            # ---- stage 1a: QT -> qt_sb ----
            qt_sb = bigp.tile([128, NT_D, D], bf16, tag="big", name="qt_sb")
            for n in range(NQ):
                for i in range(NT_D):
                    ps = psp.tile([128, 512], fp32, tag="ps")
                    for j in range(NT_D):
                        nc.tensor.matmul(
                            ps[:], wq_blks[i][:, j * 128:(j + 1) * 128],
                            xq_chs[n][:, j, :],
                            start=(j == 0), stop=(j == NT_D - 1))
                    if with_bq:
                        nc.scalar.activation(
                            qt_sb[:, i, n * 512:(n + 1) * 512], ps[:],
                            mybir.ActivationFunctionType.Identity,
                            bias=bq_sb[:, i:i + 1])
                    else:
                        nc.scalar.activation(
                            qt_sb[:, i, n * 512:(n + 1) * 512], ps[:],
                            mybir.ActivationFunctionType.Copy)

            if dist:
                # read the gathered pair back, recover the partner's block via
                # remote = g0 ^ g1 ^ local (exact bf16 bit identity) -> ktr_sb.
                # sync + gpsimd (both are past their other issue duties; the
                # collective-wait embedded in these DMAs blocks the engine,
                # so they must never sit on the scalar stream)
                for i in range(NT_D):
                    cc_view = cc_outs[i // IPG].ap().rearrange(
                        "(b r) f -> r b f", b=2)
                    r0 = (i % IPG) * 128
                    g_ch = gchp.tile([128, 2, QL], bf16, tag="gch",
                                     name=f"gch{i}")
                    eng = nc.sync if i % 2 == 0 else nc.gpsimd
                    eng.dma_start(
                        g_ch[:, 0, :], cc_view[r0:r0 + 128, 0, :])
                    eng.dma_start(
                        g_ch[:, 1, :], cc_view[r0:r0 + 128, 1, :])
                    nc.vector.tensor_tensor(
                        g_ch[:, 0, :].bitcast(uint32),
                        g_ch[:, 0, :].bitcast(uint32),
                        g_ch[:, 1, :].bitcast(uint32),
                        mybir.AluOpType.bitwise_xor)
                    nc.vector.tensor_tensor(
                        ktr_sb[:, i, :].bitcast(uint32),
                        g_ch[:, 0, :].bitcast(uint32),
                        kt_sb[:, i, :].bitcast(uint32),
                        mybir.AluOpType.bitwise_xor)

            # ---- stage 2: scores^T + exp ----
            def score_group(m, n):
                kt, mm = (ktr_sb, m - NT_L) if (dist and m >= NT_L) else (kt_sb, m)
                ps = psp.tile([128, 512], fp32, tag="ps")
                for i in range(NT_D):
                    nc.tensor.matmul(
                        ps[:], kt[:, i, mm * 128:(mm + 1) * 128],
                        qt_sb[:, i, n * 512:(n + 1) * 512],
                        start=(i == 0), stop=(i == NT_D - 1))
                if with_mask:
                    mk = maskp.tile([128, 512], bf16, tag="mask")
                    nc.sync.dma_start(
                        mk[:], maskt_d.ap()[m * 128:(m + 1) * 128,
                                            n * 512:(n + 1) * 512])
                    nc.vector.tensor_tensor(
                        ps[:], ps[:], mk[:], mybir.AluOpType.add)
                nc.scalar.activation(
                    et_sb[:, m, n * 512:(n + 1) * 512], ps[:],
                    mybir.ActivationFunctionType.Exp, scale=SCALE)

            first_ms = range(NT_L) if dist else range(NT_S)
            for n in range(NQ):
                for m in first_ms:
                    score_group(m, n)

            # softmax denominators: accumulate expT tiles on the DVE (PE
            # has no slack; DVE has plenty). In-place fp32 chain.
            sacc = cons.tile([128, QL], fp32, tag="sacc")
            first_l = list(first_ms)
            nc.vector.tensor_tensor(
                sacc[:], et_sb[:, first_l[0], :], et_sb[:, first_l[1], :],
                mybir.AluOpType.add)
            for m in first_l[2:]:
                nc.vector.tensor_tensor(
                    sacc[:], sacc[:], et_sb[:, m, :], mybir.AluOpType.add)

            # ---- stage 3a: HT over available s-tiles ----
            ht_sb = bigp.tile([128, NT_D, D], bf16, tag="big", name="ht_sb")

            def ht_groups(ms, merge, skip_prefetch=0):
                prefetch_xkv(ms, range(skip_prefetch, NT_D))
                for j in range(NT_D):
                    xkv_ch = xkv_chunks[(j, ms[0])]
                    for n in range(NQ):
                        ps = psp.tile([128, 512], fp32, tag="ps")
                        for k2, m in enumerate(ms):
                            nc.tensor.matmul(
                                ps[:], xkv_ch[:, k2, :],
                                et_sb[:, m, n * 512:(n + 1) * 512],
                                start=(k2 == 0), stop=(k2 == len(ms) - 1))
                        dst = ht_sb[:, j, n * 512:(n + 1) * 512]
                        if merge:
                            nc.vector.tensor_tensor(
                                dst, ps[:], dst, mybir.AluOpType.add)
                        else:
                            nc.scalar.activation(
                                dst, ps[:],
                                mybir.ActivationFunctionType.Copy)

            if dist:
                ht_groups(local_ms, merge=False, skip_prefetch=4)
                # m-outer: exp(m) completes both q-chunks back-to-back so the
                # DVE sums chain below never lags the PE
                for m in range(NT_L, NT_S):
                    for n in range(NQ):
                        score_group(m, n)
                for m in range(NT_L, NT_S):
                    nc.vector.tensor_tensor(
                        sacc[:], sacc[:], et_sb[:, m, :], mybir.AluOpType.add)
                ht_groups(list(range(NT_L, NT_S)), merge=True)
            else:
                ht_groups(local_ms, merge=False, skip_prefetch=4)

            # sums[1, q]: single fp32 ones-matmul per q-chunk over sacc.
            # Allocated here; EMITTED inside stage 4 after the first output
            # group so the PE chews useful matmuls while the DVE chain ends.
            sums_sb = cons.tile([1, QL], fp32, tag="sums")
            pst = pstp.tile([128, 8], fp32, tag="pst")
            recip_sb = cons.tile([128, 8], fp32, tag="recip")
            if with_bv:
                sums_bf = cons.tile([1, QL], bf16, tag="sums_bf")

            def emit_sums():
                for n in range(NQ):
                    pss = pssp.tile([1, 512], fp32, tag="pss")
                    nc.tensor.matmul(
                        pss[:], ones_col[:], sacc[:, n * 512:(n + 1) * 512],
                        start=True, stop=True)
                    nc.scalar.activation(
                        sums_sb[:, n * 512:(n + 1) * 512], pss[:],
                        mybir.ActivationFunctionType.Copy)
                for p in range(8):
                    nc.tensor.transpose(
                        pst[:, p:p + 1], sums_sb[:, p * 128:(p + 1) * 128],
                        ident1[:])
                nc.vector.reciprocal(recip_sb[:], pst[:])
                if with_bv:
                    # out accumulates UNNORMALIZED; bias enters as sums[q]*bv
                    # so the final 1/sums scale leaves exactly +bv
                    nc.scalar.activation(sums_bf[:], sums_sb[:],
                                         mybir.ActivationFunctionType.Copy)

            # ---- stage 4: out = HT^T . Wv (+bv), normalized, bf16 out ----
            # p=0: matmuls first, then the sums block (PE stays busy while
            # the DVE chain finishes), then the p=0 normalization.
            for p in range(8):
                out_sb = outp.tile([128, D], bf16, tag="outsb")
                group_ps = []
                for n2 in range(NV):
                    ps = psp.tile([128, 512], fp32, tag="ps")
                    for j in range(NT_D):
                        nc.tensor.matmul(
                            ps[:], ht_sb[:, j, p * 128:(p + 1) * 128],
                            wv_sb[:, j, n2 * 512:(n2 + 1) * 512],
                            start=(j == 0),
                            stop=(j == NT_D - 1 and not with_bv))
                    group_ps.append(ps)
                if p == 0:
                    emit_sums()
                for n2, ps in enumerate(group_ps):
                    if with_bv:
                        nc.tensor.matmul(
                            ps[:], sums_bf[:, p * 128:(p + 1) * 128],
                            bv_sb[:, n2 * 512:(n2 + 1) * 512],
                            start=False, stop=True)
                    if not (p == 7 and n2 == NV - 1):
                        nc.scalar.activation(
                            out_sb[:, n2 * 512:(n2 + 1) * 512], ps[:],
                            mybir.ActivationFunctionType.Copy,
                            scale=recip_sb[:, p:p + 1])
                    if p == 7 and n2 == NV - 1:
                        # final chunk split: 256-col act+DMA pairs on both
                        # queues so the drain after the last matmul is short
                        for hh in range(2):
                            lo = n2 * 512 + hh * 256
                            nc.scalar.activation(
                                out_sb[:, lo:lo + 256], ps[:, hh * 256:
                                                           hh * 256 + 256],
                                mybir.ActivationFunctionType.Copy,
                                scale=recip_sb[:, p:p + 1])
                            (nc.sync if hh == 0 else nc.scalar).dma_start(
                                out_d.ap()[p * 128:(p + 1) * 128, lo:lo + 256],
                                out_sb[:, lo:lo + 256])
                    else:
                        (nc.sync if (p + n2) % 2 == 0 else
                         nc.scalar).dma_start(
                            out_d.ap()[p * 128:(p + 1) * 128,
                                       n2 * 512:(n2 + 1) * 512],
                            out_sb[:, n2 * 512:(n2 + 1) * 512])

    nc.compile()
    return nc


def _get_nc(flags):
    if flags not in _cache:
        _cache[flags] = _build(*flags)
    return _cache[flags]


def _flags_of(inputs, dist=True):
    return _prep_in_maps(**inputs, dist=dist)[0]


def _pack_w(W):
    # [j*128+p, i*128+c] -> [p, i*1024 + j*128 + c]
    return np.ascontiguousarray(
        W.reshape(8, 128, 8, 128).transpose(1, 2, 0, 3).reshape(128, 8192))


def _pack_x(xt, widths):
    # xt [D, L] (row-major) -> [128, sum_c 8*w_c]: for each col-chunk c,
    # block[p, j*w + t] = xt[j*128+p, off+t]  (SBUF tile layout, so each
    # chunk is one fully-contiguous DMA)
    blocks = []
    off = 0
    for w in widths:
        blk = xt[:, off:off + w].reshape(8, 128, w).transpose(1, 0, 2)
        blocks.append(blk.reshape(128, 8 * w))
        off += w
    return np.ascontiguousarray(np.concatenate(blocks, axis=1))


def _pack_kv(xkv):
    # xkv [S, D] -> [128, j*S + m*128 + t] with xkv[m*128+p, j*128+t]:
    # per (s-pass, j) loads are fully contiguous
    nts = xkv.shape[0] // 128
    return np.ascontiguousarray(
        xkv.reshape(nts, 128, 8, 128).transpose(1, 2, 0, 3)
        .reshape(128, nts * 1024))


def _prep_in_maps(query_input, keyvalue_input, mask, Wq, bq, Wk, bk, Wv, bv,
                  dist=True):
    qi = np.asarray(query_input, np.float32)
    kv = np.asarray(keyvalue_input, np.float32)
    mask = np.asarray(mask, np.float32)
    Wqb = np.asarray(Wq, np.float32).astype(BF16)
    Wkb = np.asarray(Wk, np.float32).astype(BF16)
    Wvb = np.asarray(Wv, np.float32).astype(BF16)
    bq = np.asarray(bq, np.float32)
    bk = np.asarray(bk, np.float32)
    bv = np.asarray(bv, np.float32)

    with_mask = bool(np.any(mask != 0.0))
    with_bq = bool(np.any(bq != 0.0))
    with_bk = bool(np.any(bk != 0.0))
    with_bv = bool(np.any(bv != 0.0))
    flags = (dist, with_mask, with_bq, with_bk, with_bv)

    Wq_p = _pack_w(Wqb)
    Wk_p = _pack_w(Wkb)

    in_maps = []
    for c in range(N_CORES):
        b, h = c // 2, c % 2
        xq = qi[b, h * QL:(h + 1) * QL, :].astype(BF16)       # [QL, D]
        xkv = kv[b].astype(BF16)                               # [S, D]
        if dist:
            xkvt = np.ascontiguousarray(xkv[h * QL:(h + 1) * QL, :].T)
            perm_kv = np.concatenate(
                [xkv[h * QL:(h + 1) * QL], xkv[(1 - h) * QL:(2 - h) * QL]])
        else:
            xkvt = np.ascontiguousarray(xkv.T)
            perm_kv = xkv
        m = {
            "xqt": _pack_x(np.ascontiguousarray(xq.T), [512, 512]),
            "xkvt": _pack_x(xkvt, _kch(xkvt.shape[1])),
            "xkv": _pack_kv(np.ascontiguousarray(perm_kv)),
            "wq": Wq_p, "wk": Wk_p, "wv": Wvb,
        }
        if with_bq:
            m["bq"] = np.ascontiguousarray(bq.reshape(8, 128).T)
        if with_bk:
            m["bk"] = np.ascontiguousarray(bk.reshape(8, 128).T)
        if with_bv:
            m["bv"] = bv.astype(BF16).reshape(1, D)
        if with_mask:
            mt = mask[b, h * QL:(h + 1) * QL, :].T * np.float32(np.sqrt(D))
            if dist:
                mt = np.concatenate(
                    [mt[h * QL:(h + 1) * QL], mt[(1 - h) * QL:(2 - h) * QL]])
            m["maskt"] = np.ascontiguousarray(mt.astype(np.float32)).astype(BF16)
        in_maps.append(m)
    return flags, in_maps


def _ensure_axon_hooks_stub():
    # bass_utils imports antenv.axon_hooks when tracing is requested (even via
    # a stray BASS_TRACE env var); the module is absent on some images, so
    # register a no-op stub if needed.
    import sys, types
    try:
        import antenv.axon_hooks  # noqa: F401
    except ImportError:
        stub = types.ModuleType("antenv.axon_hooks")
        stub._hook = None
        stub.set_axon_ntff_profile_hook = (
            lambda h: setattr(stub, "_hook", h))
        stub.get_axon_ntff_profile_hook = lambda: stub._hook
        sys.modules["antenv.axon_hooks"] = stub
        try:
            import antenv
            antenv.axon_hooks = stub
        except ImportError:
            pass


def _run(inputs, trace=False, **kw):
    _ensure_axon_hooks_stub()
    from concourse import bass_utils
    dist = os.environ.get("KERNEL_DIST", "1") == "1"
    ps_bufs = int(os.environ.get("KERNEL_PSBUFS", "5"))
    flags, in_maps = _prep_in_maps(**inputs, dist=dist)
    nc = _get_nc(flags + (ps_bufs,))
    res = bass_utils.run_bass_kernel_spmd(
        nc, in_maps, core_ids=list(range(N_CORES)), trace=trace, **kw)
    out = np.empty((B, S, D), np.float32)
    for c in range(N_CORES):
        b, h = c // 2, c % 2
        out[b, h * QL:(h + 1) * QL, :] = np.asarray(
            res.results[c]["out"], dtype=np.float32)
    return out, res


def kernel(**inputs) -> np.ndarray:
    out, _ = _run(inputs, trace=False)
    return out
